# revision 65
# baseline (speedup 1.0000x reference)
"""Trainium2 Bass kernel for batched causal multi-head attention.

Problem: x[B=8,S=1024,D=768], per-head projections W_Q/W_K/W_V [H=12,D,DH=64],
W_O [H,DH,D]; causal softmax attention; output [B,S,D].

Strategy: data-parallel over batch across 8 NeuronCores (no collectives).
Per core (one batch element), computed fully on-chip:
  - qT/kT per head-pair in [e, s] layout via W-stationary matmuls (row-tiled
    K=64 head pairs run concurrently in the PE array).
  - scores^T [j, i] tiles = kT.T @ qT directly; causal block-skipping; exp on
    ScalarE (scale=1/8 folded in); triangular-block mask via a 0/1 mask mult.
  - z^T = (v | ones).T @ p^T accumulated over j-tiles in PSUM; the ones column
    yields the softmax denominator as row 64 (no extra matmul).
  - normalization is interleaved into phase B with a one-pair delay: the raw
    den row is cast to bf16 in place at partition 64, two 1-row selector
    matmuls broadcast it to 128 partitions, the reciprocal runs on the
    broadcast at partition base 0 (custom-DVE ucode computes garbage at
    other bases), and one DVE multiply normalizes z in SBUF. No den DMA and
    no phase-C stall; O-proj PSUM reuses the ps_z slots so no pool boundary
    serializes phase C behind the last normalization read.
  - output projection accumulates head pairs with K=128 stacked lhsT.
Matmul operands are bf16 (full PE rate at any moving-dim; scores accumulate in
fp32 PSUM so exp sees unrounded scores). Host-side prep (free): transpose/pack
x and weights into exact bf16 SBUF images. `reps`/`loop_reps`/`phases` are
benchmarking aids (static unroll / on-device For_i loop / phase subsetting).

Schedule notes (measured on HW via NTFF traces): the Tile list-scheduler
software-pipelines the qk projections ~20us ahead, so the per-pair critical
path is scores->exp->z; the normalization chain must stay off the tensor
queue's ready-frontier (bc slots recycle into the next pair's scores
matmuls). tc.high_priority on ops with DMA dependencies corrupts results
(semaphore targets derive from scheduled position) - do not use it here.
"""

import os
from contextlib import ExitStack

import numpy as np

B, S, D, H, DH = 8, 1024, 768, 12, 64
P = 128
DT = 6  # d tiles (D / 128)
ST = 8  # s tiles (S / 128)
PAIRS = 6  # head pairs (H / 2)
NB = 512  # i-block width
SCALE = 1.0 / 8.0  # 1/sqrt(DH)

_CACHE = {}


def _build(qk_bias: bool, v_bias: bool, reps: int = 1, loop_reps: int = 0, phases: str = 'abc'):
    import concourse.bass as bass  # noqa: F401
    import concourse.mybir as mybir
    import concourse.tile as tile
    from concourse import bacc

    f32 = mybir.dt.float32
    bf16 = mybir.dt.bfloat16
    Exp = mybir.ActivationFunctionType.Exp

    nc = bacc.Bacc("TRN2", target_bir_lowering=False, debug=False)

    xT = nc.dram_tensor("xT", [P, DT, S], bf16, kind="ExternalInput").ap()
    wq = nc.dram_tensor("wq", [P, PAIRS, DT, P], bf16, kind="ExternalInput").ap()
    wk = nc.dram_tensor("wk", [P, PAIRS, DT, P], bf16, kind="ExternalInput").ap()
    wv = nc.dram_tensor("wv", [P, DT, D], bf16, kind="ExternalInput").ap()
    wo = nc.dram_tensor("wo", [P, PAIRS, D], bf16, kind="ExternalInput").ap()
    mask2 = nc.dram_tensor("mask2", [P, 2, P], bf16, kind="ExternalInput").ap()
    selr = nc.dram_tensor("selr", [P, P], bf16, kind="ExternalInput").ap()
    if qk_bias:
        bq = nc.dram_tensor("bq", [P, PAIRS], f32, kind="ExternalInput").ap()
        bk = nc.dram_tensor("bk", [P, PAIRS], f32, kind="ExternalInput").ap()
    if v_bias:
        bv = nc.dram_tensor("bv", [1, D], f32, kind="ExternalInput").ap()
    out = nc.dram_tensor("out", [S, D], bf16, kind="ExternalOutput").ap()

    def mmr(o, lhsT, rhs, start, stop):
        nc.tensor.matmul(o, lhsT, rhs, start=start, stop=stop)

    with tile.TileContext(nc) as tc:
      with ExitStack() as loop_ctx:
        if loop_reps:
            loop_ctx.enter_context(tc.For_i(0, loop_reps, 1))
        for _rep in range(reps):
          with ExitStack() as ctx:
            consts = ctx.enter_context(tc.tile_pool(name="consts", bufs=1))
            xt_p = ctx.enter_context(tc.tile_pool(name="xt", bufs=1))
            w_p = ctx.enter_context(tc.tile_pool(name="w", bufs=1))
            v_p = ctx.enter_context(tc.tile_pool(name="v", bufs=1))
            z_p = ctx.enter_context(tc.tile_pool(name="z", bufs=1))
            qk_p = ctx.enter_context(tc.tile_pool(name="qk", bufs=3))
            p_p = ctx.enter_context(tc.tile_pool(name="p", bufs=6))
            rec_p = ctx.enter_context(tc.tile_pool(name="rec", bufs=6))
            out_p = ctx.enter_context(tc.tile_pool(name="out", bufs=3))

            # DMA order + chunking: xt/wv gate the first v-proj matmuls, so
            # land them in fine-grained pieces (Tile deps are AP-range aware);
            # wq/wk per pair; wo/mask are needed much later.
            xt = xt_p.tile([P, DT, S], bf16)
            wv_t = w_p.tile([P, DT, D], bf16, tag="wv")
            # smallest-possible first chunks: the very first vp1 matmul needs
            # only wv dt0 cols 0:NB and the dt0 slice of xt st0 (160KB
            # instead of 324KB before the PE can start)
            nc.sync.dma_start(out=wv_t[:, 0:1, 0:NB], in_=wv[:, 0:1, 0:NB])
            nc.sync.dma_start(out=xt[:, 0:1, 0:P], in_=xT[:, 0:1, 0:P])
            nc.sync.dma_start(out=xt[:, 1:DT, 0:P], in_=xT[:, 1:DT, 0:P])
            nc.sync.dma_start(out=wv_t[:, 0:1, NB:D], in_=wv[:, 0:1, NB:D])
            for dt in range(1, DT):
                nc.sync.dma_start(out=wv_t[:, dt : dt + 1, :], in_=wv[:, dt : dt + 1, :])
            for st in range(1, ST):
                nc.sync.dma_start(
                    out=xt[:, :, st * P : (st + 1) * P],
                    in_=xT[:, :, st * P : (st + 1) * P],
                )
            wq_t = w_p.tile([P, PAIRS, DT, P], bf16, tag="wq")
            wk_t = w_p.tile([P, PAIRS, DT, P], bf16, tag="wk")
            for pr in range(PAIRS):
                nc.sync.dma_start(
                    out=wq_t[:, pr : pr + 1, :, :], in_=wq[:, pr : pr + 1, :, :]
                )
                nc.sync.dma_start(
                    out=wk_t[:, pr : pr + 1, :, :], in_=wk[:, pr : pr + 1, :, :]
                )
            mask2_t = consts.tile([P, 2, P], bf16)
            nc.sync.dma_start(out=mask2_t[:, :, :], in_=mask2[:, :, :])
            wo_t = w_p.tile([P, PAIRS, D], bf16, tag="wo")
            nc.sync.dma_start(out=wo_t[:, :, :], in_=wo[:, :, :])
            if qk_bias:
                bq_t = consts.tile([P, PAIRS], f32, tag="bq")
                nc.sync.dma_start(out=bq_t[:, :], in_=bq[:, :])
                bk_t = consts.tile([P, PAIRS], f32, tag="bk")
                nc.sync.dma_start(out=bk_t[:, :], in_=bk[:, :])
            if v_bias:
                bv_row = consts.tile([P, D], f32, tag="bvr")
                nc.sync.dma_start(out=bv_row[0:1, :], in_=bv[:, :])
                bv_full = consts.tile([P, D], f32, tag="bvf")
                nc.gpsimd.partition_broadcast(bv_full[:, :], bv_row[0:1, :])

            # v layout: [s-tile, head, 66] — even heads: col 64 = 1.0 (den
            # lands at PSUM row 64); odd heads: col 64 = 0.0, col 65 = 1.0
            # (den lands at row 65). With the two dens at DIFFERENT
            # partitions, one c=2 selector matmul broadcasts both quadrants.
            v_t = v_p.tile([P, ST, H, DH + 2], bf16)
            if 'a' in phases:
                for st in range(ST):
                    for h in range(H):
                        nc.vector.memset(
                            v_t[:, st, h : h + 1, DH],
                            1.0 if h % 2 == 0 else 0.0,
                        )
                        if h % 2 == 1:
                            nc.vector.memset(v_t[:, st, h : h + 1, DH + 1], 1.0)
            else:
                nc.vector.memset(v_t[:, :, :, :], 1.0)

            z_t = z_p.tile([P, PAIRS, S], bf16)
            if 'b' not in phases:
                nc.vector.memset(z_t[:, :, :], 0.0)
            # unnormalized-z denominators: head even at partition 0, head odd
            # at partition 32 (DMA start partitions must be 32-aligned); slot
            # g=(pr,ib). Unused rows stay 1.0 so the batched reciprocal is
            # finite (they are zeroed by the selector matmul anyway).
            # Softmax denominators never leave partition 64: the reciprocal
            # reads the z-matmul's ones-row straight out of PSUM at partition
            # 64, the bf16 cast stays there, and a 1-row selector matmul
            # (lhsT/rhs base partition 64) broadcasts 1/den into a [128, NB]
            # PSUM tile quadrant-by-quadrant. No den DMA, no staging copy —
            # the whole chain is ~1.7us instead of ~5us.
            # selector (host-built, DMA'd: row-65 writes are not 32-aligned
            # for DVE memsets): row 64 -> out 0:64, row 65 -> out 64:128,
            # everything else zero (the PE rounds the c=2 stationary to a
            # 32-row band, so rows 66..95 must be finite zeros).
            sel2r = consts.tile([P, P], bf16, tag="sel2r")
            nc.sync.dma_start(out=sel2r[:, :], in_=selr[:, :])
            # den rows per (pair, i-block): both heads' dens in one tile
            rbf_g = {}

            # ---------------- Phase A: V projection (all heads) ------------
            with tc.tile_pool(name="ps_v", bufs=2, space="PSUM") as ps_v:
              if 'a' in phases:
                  for st in range(ST):
                      vp1 = ps_v.tile([P, NB], f32, tag="v1")
                      vp2 = ps_v.tile([P, D - NB], f32, tag="v2")
                      # dt-outer with vp1/vp2 paired: consecutive matmuls
                      # share the same stationary (xt tile) so the weight
                      # load can be elided/cached by codegen.
                      for dt in range(DT):
                          lhsT = xt[:, dt, st * P : (st + 1) * P]
                          mmr(vp1[:, :], lhsT, wv_t[:, dt, 0:NB], dt == 0, dt == DT - 1)
                          mmr(vp2[:, :], lhsT, wv_t[:, dt, NB:D], dt == 0, dt == DT - 1)
                      # vp1 copy on scalar, the smaller vp2 copy on vector:
                      # halves scalar's phase-A load so the hoisted pair-0/1
                      # q/k copies run during A (they gated the A->B seam).
                      # Vector's only backlog then is ~2.4us of v_t memsets,
                      # done before st2's slot-WAR needs the st0 copy.
                      nc.scalar.copy(
                          v_t[:, st, 0:8, 0:DH],
                          vp1.rearrange("p (h e) -> p h e", e=DH),
                      )
                      nc.vector.tensor_copy(
                          v_t[:, st, 8:12, 0:DH],
                          vp2.rearrange("p (h e) -> p h e", e=DH),
                      )
                      if v_bias:
                          nc.vector.tensor_add(
                              v_t[:, st, :, 0:DH],
                              v_t[:, st, :, 0:DH],
                              bv_full.rearrange("p (h e) -> p h e", e=DH),
                          )

            # rbf slot pre-zero emitted AFTER phase A: always-ready memsets
            # emitted earlier would sit ahead of the phase-A copies in the
            # vector priority queue and stall the ps_v slot recycling (the
            # v14 regression); here they are filler, needed only by pair 0's
            # first den cast (~33us in).
            for _ in range(4):
                rbf0 = rec_p.tile([P, NB], bf16, tag="rbf", bufs=4, name="rbf0")
                nc.vector.memset(rbf0[:, :], 0.0)

            # ---------------- Phase B: per head-pair attention --------------
            with (
                tc.tile_pool(name="ps_qk", bufs=2, space="PSUM") as ps_qk,
                tc.tile_pool(name="ps_sc", bufs=2, space="PSUM") as ps_sc,
                tc.tile_pool(name="ps_z", bufs=2, space="PSUM") as ps_z,
            ):
                # Normalization is interleaved into phase B with a one-pair
                # delay: rec (vector) right after the den DMA lands; the
                # selector-matmul broadcast + z multiply at the start of the
                # NEXT pair (so the tensor queue never waits on the vector
                # queue). bc reuses the ps_sc slots — by the time a slot is
                # recycled its exp reader is long done, and the pair's first
                # scores matmul only lands after its 6.4us qk projection.
                nf_mul_pending = []

                def emit_norm_finish(pr, ib):
                    # Broadcast the raw bf16 den from partition 64 to all 128
                    # partitions via two 1-row selector matmuls, THEN take the
                    # reciprocal of the broadcast at partition base 0 —
                    # custom-DVE ucode ops (reciprocal_approx) compute garbage
                    # at partition bases != 0.
                    # bc lives in the ps_sc slots. (Borrowing ps_qk slots
                    # instead blocks the scheduler's ~20us-ahead hoisting of
                    # the q/k projections and costs ~25us of tensor time.)
                    # One c=2 selector matmul broadcasts both heads' dens
                    # (rows 64/65) into the two out quadrants.
                    # The z multiply is DEFERRED one nf-point (emitted via
                    # nf_mul_pending below): its result is only needed by the
                    # O-projection, but emitted here it would outrank the
                    # next block's den casts in the vector priority queue and
                    # stretch the pair-boundary chain by ~0.7us.
                    # NOTE: high_priority on the mul corrupts the result (the
                    # hoisted mul races its z-DMA dependency; semaphore
                    # targets are computed from scheduled position).
                    g = 2 * pr + ib
                    bc = ps_sc.tile([P, 2, NB], f32, tag="sc", name="bcn")
                    nc.tensor.matmul(
                        bc[:, 0, :],
                        sel2r[DH : DH + 2, :],
                        rbf_g[g][DH : DH + 2, :],
                        start=True, stop=True,
                    )
                    rec_sb = rec_p.tile([P, NB], f32, tag="recsb", bufs=2)
                    nc.vector.reciprocal_approx_fast(rec_sb[:, :], bc[:, 0, :])
                    nf_mul_pending.append((pr, ib, rec_sb))

                def emit_nf_muls():
                    while nf_mul_pending:
                        mpr, mib, msb = nf_mul_pending.pop(0)
                        nc.vector.tensor_mul(
                            z_t[:, mpr, mib * NB : (mib + 1) * NB],
                            z_t[:, mpr, mib * NB : (mib + 1) * NB],
                            msb[:, :],
                        )

                for pr in range(PAIRS if 'b' in phases else 0):
                    qT_t = qk_p.tile([P, S], bf16, tag="q")
                    kT_t = qk_p.tile([P, S], bf16, tag="k")

                    def emit_proj(dst, w_t, b_t):
                        for ib in range(2):
                            ps = ps_qk.tile([P, NB], f32, tag="qk", name="ps")
                            for dt in range(DT):
                                mmr(
                                    ps[:, :],
                                    w_t[:, pr, dt, :],
                                    xt[:, dt, ib * NB : (ib + 1) * NB],
                                    dt == 0,
                                    dt == DT - 1,
                                )
                            nc.scalar.copy(dst[:, ib * NB : (ib + 1) * NB], ps[:, :])
                        if qk_bias:
                            bias_ap = (bq_t if b_t == "bq" else bk_t)[:, pr : pr + 1]
                            nc.vector.tensor_scalar_add(dst[:, :], dst[:, :], bias_ap)

                    # norm-finish(pr-1, ib0) before the q-proj; ib1's is
                    # emitted between the two i-block loops below, so its
                    # bc-slot WAR and den-chain latency land where the PE
                    # has a full i-block of ready scores work queued.
                    if pr > 0 and 'n' not in phases:
                        emit_nf_muls()
                        emit_norm_finish(pr - 1, 0)
                    emit_proj(qT_t, wq_t, "bq")
                    emit_proj(kT_t, wk_t, "bk")

                    for ib in range(2):
                        if ib == 1 and pr > 0 and 'n' not in phases:
                            emit_nf_muls()
                            emit_norm_finish(pr - 1, 1)
                        njt = 4 * (ib + 1)
                        zps = [
                            ps_z.tile([DH + 2, NB], f32, tag="z", name="zpsA"),
                            ps_z.tile([DH + 2, NB], f32, tag="z", name="zpsB"),
                        ]
                        def emit_z(jt, pt, o):
                            for h2 in range(2):
                                h = 2 * pr + h2
                                w = DH + 1 + h2  # odd head: extra zero col,
                                # den rides at out row 65 instead of 64
                                mmr(
                                    zps[h2][0:w, o:NB],
                                    v_t[:, jt, h, 0:w],
                                    pt[:, h2, o:NB],
                                    jt == 0,
                                    jt == njt - 1,
                                )

                        # staggered: z-matmul for tile jt-1 is emitted after the
                        # scores matmul of tile jt, so the in-order PE never
                        # stalls on the exp+mask latency of the current tile.
                        prev = None
                        for jt in range(njt):
                            o = max(0, P * jt - NB * ib)
                            sps = ps_sc.tile([P, 2, NB], f32, tag="sc")
                            for h2 in range(2):
                                mmr(
                                    sps[:, h2, o:NB],
                                    kT_t[64 * h2 : 64 * (h2 + 1), jt * P : (jt + 1) * P],
                                    qT_t[64 * h2 : 64 * (h2 + 1), ib * NB + o : (ib + 1) * NB],
                                    True,
                                    True,
                                )
                            pt = p_p.tile([P, 2, NB], bf16, tag="p")
                            nc.scalar.activation(
                                pt[:, :, o:NB], sps[:, :, o:NB], Exp, scale=SCALE
                            )
                            if P * jt - NB * ib >= 0:  # diagonal crossing tile
                                nc.vector.tensor_mul(
                                    pt[:, :, o : o + P],
                                    pt[:, :, o : o + P],
                                    mask2_t[:, :, :],
                                )
                            if prev is not None:
                                emit_z(*prev)
                            prev = (jt, pt, o)
                        emit_z(*prev)
                        g = 2 * pr + ib
                        if 'n' not in phases:
                            rbf = rec_p.tile([P, NB], bf16, tag="rbf", bufs=4)
                            rbf_g[g] = rbf
                        if 'n' not in phases:
                            # raw den rows -> bf16 staging at partitions
                            # 64/65 of one tile. DVE partition bases must be
                            # 32-aligned, so the odd head's [64:66] (zero row
                            # + den) goes first and the even head's den then
                            # overwrites row 64. On vector — scalar looks
                            # idle at pair ends but owes the hoisted
                            # next-pair q/k copies (using it cost 18us).
                            nc.vector.tensor_copy(
                                rbf[DH : DH + 2, :],
                                zps[1][DH : DH + 2, :],
                            )
                            nc.vector.tensor_copy(
                                rbf[DH : DH + 1, :],
                                zps[0][DH : DH + 1, :],
                            )
                        for h2 in range(2):
                            # one DVE cast moves unnormalized z to bf16 SBUF;
                            # z then moves partitions via DMA (normalization
                            # is finished one pair later, interleaved).
                            ztmp = rec_p.tile([64, NB], bf16, tag="ztmp")
                            nc.vector.tensor_copy(ztmp[:, :], zps[h2][0:64, :])
                            nc.sync.dma_start(
                                z_t[64 * h2 : 64 * (h2 + 1), pr, ib * NB : (ib + 1) * NB],
                                ztmp[:, :],
                            )

                if 'b' in phases and 'n' not in phases:
                    emit_nf_muls()
                    emit_norm_finish(PAIRS - 1, 0)
                    emit_norm_finish(PAIRS - 1, 1)
                    emit_nf_muls()

                # ---------- Phase C: output projection --------------------
                # O-proj PSUM comes from the ps_z pool (tag "z") so phase C
                # needs no new pool: a fresh pool's alloc boundary would
                # wait for ALL phase-B PSUM readers (incl. the last bc mul),
                # idling the PE ~5us before the first O matmul.
                for st in range(ST if 'c' in phases else 0):
                    op1 = ps_z.tile([P, NB], f32, tag="z", name="op1")
                    op2 = ps_z.tile([P, D - NB], f32, tag="z", name="op2")
                    # paired: op1/op2 share the stationary z tile per pr
                    for pr in range(PAIRS):
                        lhsT = z_t[:, pr, st * P : (st + 1) * P]
                        mmr(op1[:, :], lhsT, wo_t[:, pr, 0:NB], pr == 0, pr == PAIRS - 1)
                        mmr(op2[:, :], lhsT, wo_t[:, pr, NB:D], pr == 0, pr == PAIRS - 1)
                    ot = out_p.tile([P, D], bf16, tag="ot")
                    nc.scalar.copy(ot[:, 0:NB], op1[:, :])
                    nc.vector.tensor_copy(ot[:, NB:D], op2[:, :])
                    # two half-DMAs: the first leaves as soon as the scalar
                    # copy lands, shortening the post-last-matmul tail
                    nc.sync.dma_start(
                        out[st * P : (st + 1) * P, 0:NB], ot[:, 0:NB]
                    )
                    nc.sync.dma_start(
                        out[st * P : (st + 1) * P, NB:D], ot[:, NB:D]
                    )

    nc.compile()
    return nc


def _pack_host(inputs):
    import ml_dtypes

    bf = ml_dtypes.bfloat16
    x = np.ascontiguousarray(np.asarray(inputs["normalized_resid_pre"], np.float32))
    WQ = np.asarray(inputs["W_Q"], np.float32)
    WK = np.asarray(inputs["W_K"], np.float32)
    WV = np.asarray(inputs["W_V"], np.float32)
    WO = np.asarray(inputs["W_O"], np.float32)

    def pack_qk(W):
        img = np.empty((P, PAIRS, DT, P), np.float32)
        for pr in range(PAIRS):
            for dt in range(DT):
                img[:, pr, dt, 0:64] = W[2 * pr, dt * P : (dt + 1) * P, :]
                img[:, pr, dt, 64:128] = W[2 * pr + 1, dt * P : (dt + 1) * P, :]
        return np.ascontiguousarray(img)

    wq_img = pack_qk(WQ).astype(bf)
    wk_img = pack_qk(WK).astype(bf)
    # wv_sb[p, dt, n] = WV_flat[dt*128+p, n];  WV_flat[d, h*64+e] = WV[h, d, e]
    wv_flat = WV.transpose(1, 0, 2).reshape(D, D)
    wv_img = np.ascontiguousarray(wv_flat.reshape(DT, P, D).transpose(1, 0, 2)).astype(
        bf
    )
    # wo_sb[p, pr, n]: rows stack the pair's two heads' DH dims
    wo_img = np.ascontiguousarray(WO.reshape(PAIRS, P, D).transpose(1, 0, 2)).astype(bf)
    m = (np.arange(P)[:, None] <= np.arange(P)[None, :]).astype(np.float32)
    mask2_img = np.ascontiguousarray(np.stack([m, m], axis=1)).astype(bf)
    # selector rows for the den broadcast: row 64 -> cols 0:64, row 65 ->
    # cols 64:128 (see kernel: one c=2 matmul broadcasts both heads' dens)
    sel_img = np.zeros((P, P), np.float32)
    sel_img[DH, 0:64] = 1.0
    sel_img[DH + 1, 64:128] = 1.0
    sel_img = sel_img.astype(bf)
    xT_imgs = [
        np.ascontiguousarray(x[b].T.reshape(DT, P, S).transpose(1, 0, 2)).astype(bf)
        for b in range(B)
    ]
    return xT_imgs, wq_img, wk_img, wv_img, wo_img, mask2_img, sel_img


def kernel(**inputs):
    global LAST_EXEC_TIME_NS
    from concourse.bass_utils import run_bass_kernel_spmd

    bq_np = np.asarray(inputs["b_Q"], np.float32)
    bk_np = np.asarray(inputs["b_K"], np.float32)
    bv_np = np.asarray(inputs["b_V"], np.float32)
    bo_np = np.asarray(inputs["b_O"], np.float32)
    qk_bias = bool(np.any(bq_np) or np.any(bk_np))
    v_bias = bool(np.any(bv_np))

    reps = int(os.environ.get("KERNEL_REPS", "1"))
    key = (qk_bias, v_bias, reps)
    if key not in _CACHE:
        _CACHE[key] = _build(qk_bias, v_bias, reps)
    nc = _CACHE[key]

    xT_imgs, wq_img, wk_img, wv_img, wo_img, mask2_img, sel_img = _pack_host(
        inputs
    )

    common = {
        "wq": wq_img,
        "wk": wk_img,
        "wv": wv_img,
        "wo": wo_img,
        "mask2": mask2_img,
        "selr": sel_img,
    }
    if qk_bias:
        common["bq"] = np.ascontiguousarray(bq_np.reshape(PAIRS, P).T)
        common["bk"] = np.ascontiguousarray(bk_np.reshape(PAIRS, P).T)
    if v_bias:
        common["bv"] = np.ascontiguousarray(bv_np.reshape(1, D))

    in_maps = [dict(common, xT=xT_imgs[b]) for b in range(B)]

    trace = os.environ.get("KERNEL_TRACE", "0") == "1"
    try:
        res = run_bass_kernel_spmd(
            nc, in_maps, core_ids=list(range(B)), trace=trace
        )
    except ModuleNotFoundError:
        # axon NTFF profiling hook unavailable in this container
        res = run_bass_kernel_spmd(nc, in_maps, core_ids=list(range(B)))
    LAST_EXEC_TIME_NS = res.exec_time_ns
    if trace and res.exec_time_ns is not None:
        print(f"HW exec time: {res.exec_time_ns} ns")

    out = np.stack(
        [np.asarray(res.results[b]["out"], np.float32) for b in range(B)], axis=0
    )
    out = out + bo_np[None, None, :]
    return out.astype(np.float32)


LAST_EXEC_TIME_NS = None

